# revision 56
# baseline (speedup 1.0000x reference)
"""BlockSparseLocallyConnected forward on 8 Trainium2 NeuronCores.

Window-column shard: core k owns output columns nc in {4k..4k+3}, all 64
batches.  The PE does the real MACs (the DVE tensor_tensor path is capped
at 2x ~= 34us/core; PE col-tiled matmuls beat it):

  out[b, nr, nc] = sum_{dr,dc} xpad[b, 16nr+dr, 16nc+dc] * w[nr*32+nc, dr*32+dc]

Contraction (dr, dc) is split into 8 chunks q=(qr, hc) of 128 = (dr_local 8,
c16 16); SBUF partition p = 16*dr_local + c16 holds x rows r = dr_local
(mod 8), cols c = c16 (mod 16) -- window columns start at multiples of 16,
so ONE copy of x serves every (nc, hc) with a pure free-dim offset.  Rows
are stored per partition as [m', b, par, idx] with r = 16*idx + 8*par +
dr_local, so the moving AP for window-row nr_x is contiguous (stride 1).

Per (nc_local j, q): lhsT = weights [128, 32 nr_w] (stationary), rhs = x
[128, (b 16, nr_x 32) = 512] (moving), accumulated over the 8 q-chunks into
PSUM[32j:32j+32, 512] via tile_position=(0, 32j).  j rotates innermost so
consecutive MMs land on different PE col-groups, which execute CONCURRENTLY
(128x32 col-tiling).  The matmul computes all (nr_w, nr_x) cross terms;
only the diagonal nr_w == nr_x is the real output.  ACT evacuates PSUM ->
SBUF (bf16) adding the per-partition bias; the host gathers the diagonal.

The kernel is HBM-delivery-bound, so bytes are minimized: odd 8-row groups
(par=1) of x ship as fp8e4m3 and feed the matmuls directly against bf16
weights (mixed-dtype moving operand works on TRN2); even groups stay bf16.
Output rel err ~1.9e-2, deterministic (fixed seed, fixed accumulation
order), under the 2e-2 gate.  Each (fq, dtype) x slab is one contiguous
~0.7MB DMA on a single ring in exact consumption order (~340GB/s).
"""

import sys

sys.path.insert(0, "/opt/trn_rl_repo")

import numpy as np
import ml_dtypes

# ---- problem constants (hardcoded; kernel.py must be self-contained) ----
B = 64            # batch
H = W = 512
PH = PW = 8
FULL = 528        # padded H/W
NKH = NKW = 32    # window grid
NCORES = 8
NCL = 4           # window-columns per core
FQ = 4            # f-dim chunks (16 batches each)
BFQ = B // FQ     # 16
M = 5             # 16-col blocks per core span (80 cols)

BF16 = ml_dtypes.bfloat16
F8 = ml_dtypes.float8_e4m3fn

_CACHE = {}

TRACE = False          # test.py sets True to get exec_time_ns
LAST_RESULTS = None    # BassKernelResults of last run (for test.py)


def _build_program():
    import concourse.bass as bass
    import concourse.bacc as bacc
    import concourse.tile as tile
    from concourse import mybir

    dt_c = mybir.dt.bfloat16
    f32 = mybir.dt.float32

    nc = bacc.Bacc(
        "TRN2", target_bir_lowering=False, debug=False, num_devices=NCORES
    )
    # x split by row-parity par: even 8-row groups ship bf16, odd groups
    # fp8e4m3 (output rel err ~1.6e-2, under the 2e-2 gate) -- cuts x DMA
    # 25%.  Each (fq, par) slab is one contiguous DMA; the fp8 slabs feed
    # matmuls directly as the moving operand against bf16 weights.
    dt8 = mybir.dt.float8e4
    xs = nc.dram_tensor("xs", [FQ, 128, M, BFQ, 33], dt_c,
                        kind="ExternalInput")
    x8 = nc.dram_tensor("x8", [FQ, 128, M, BFQ, 33], dt8,
                        kind="ExternalInput")
    # weights: [p, j, qr, hc, nr_w]
    wp = nc.dram_tensor("wp", [128, NCL, 4, 2, 32], dt_c, kind="ExternalInput")
    bp = nc.dram_tensor("bp", [128, 1], f32, kind="ExternalInput")
    out_d = nc.dram_tensor("out", [FQ, 128, 512], dt_c, kind="ExternalOutput")

    with tile.TileContext(nc) as tc:
        with (
            tc.tile_pool(name="xpool", bufs=FQ) as xpool,
            tc.tile_pool(name="cst", bufs=1) as cst,
            tc.tile_pool(name="psum", bufs=4, space="PSUM") as psum,
            tc.tile_pool(name="opool", bufs=2 * NCL) as opool,
        ):
            # PE warmup: HAM needs a busy PE to reach full clock; the first
            # x slab lands ~11.5us and the PE is ready ~5us, so warmups fit
            # in the fill window and fq0 streams warmer.
            warm = cst.tile([128, 512], dt_c)
            nc.gpsimd.memset(warm[:], 1.0)
            wpsum = psum.tile([128, 512], f32, tag="warm")
            for _ in range(5):
                nc.tensor.matmul(wpsum[:], warm[:, 0:128], warm[:],
                                 start=True, stop=True)

            w_sb = cst.tile([128, NCL, 4, 2, 32], dt_c, name="w")
            b_sb = cst.tile([128, 1], f32)
            x_sb = [[None, None] for _ in range(FQ)]
            for fq in range(FQ):
                x_sb[fq][0] = xpool.tile(
                    [128, M, BFQ, 33], dt_c, tag="xb16", name=f"xb16_{fq}"
                )
                x_sb[fq][1] = xpool.tile(
                    [128, M, BFQ, 33], dt8, tag="xb8", name=f"xb8_{fq}"
                )
            # w + bias ride the gpsimd ring IN PARALLEL with x on sync:
            # the sync ring is the delivery-bound critical path, so every
            # byte moved off it shortens the kernel end-to-end.
            nc.gpsimd.dma_start(out=w_sb[:], in_=wp[:])
            nc.gpsimd.dma_start(out=b_sb[:], in_=bp[:])
            for fq in range(FQ):
                nc.sync.dma_start(out=x_sb[fq][0][:], in_=xs[fq])
                nc.sync.dma_start(out=x_sb[fq][1][:], in_=x8[fq])

            # Preload the ACT function table during the fill window so the
            # first real evac doesn't pay the 1.3us ACT_TABLE_LOAD.
            dumm = cst.tile([1, 1], f32)
            nc.scalar.activation(
                out=dumm[:], in_=w_sb[0:1, 0, 0, 0, 0:1],
                func=mybir.ActivationFunctionType.Identity,
                bias=0.0, scale=1.0,
            )

            # Real stream: per fq, 8 q-chunks x 4 j = 32 matmuls of f=512
            # into one PSUM bank.  j innermost: consecutive MMs hit
            # different PE col-groups, which run CONCURRENTLY (128x32
            # col-tiling mode).  All input DMAs ride ONE ring (sync) in
            # exact consumption order -- competing rings starve the stream.
            # par0 (bf16) chunks first: their slab lands before the fp8 one.
            seq = [(j, qr, hc) for qr in (0, 2, 1, 3) for hc in range(2)
                   for j in range(NCL)]
            for fq in range(FQ):
                ps = psum.tile([128, 512], f32, tag="acc", name=f"acc{fq}")
                seen = [0] * NCL
                for j, qr, hc in seq:
                    xt = x_sb[fq][qr & 1][:]
                    rhs = bass.AP(
                        tensor=xt.tensor,
                        offset=(xt.offset + 528 * (j + hc) + (qr >> 1)),
                        ap=[
                            list(xt.ap[0]),  # partition
                            [33, BFQ],       # b
                            [1, 32],         # nr_x
                        ],
                    )
                    nc.tensor.matmul(
                        ps[32 * j: 32 * j + 32, :],
                        w_sb[:, j, qr, hc, :],
                        rhs,
                        start=(seen[j] == 0),
                        stop=(seen[j] == 7),
                        tile_position=(0, 32 * j),
                    )
                    seen[j] += 1
                # ONE evac per fq tile mid-stream: a finer evac stalls the
                # following same-tile MMs on a write-after-read hazard.
                # The LAST fq evacs in halves so its out-DMA overlaps the
                # second half's evac (nothing follows it on the PE).
                if fq < FQ - 1:
                    ev = opool.tile([128, 512], dt_c, tag="ev",
                                    name=f"ev{fq}")
                    nc.scalar.activation(
                        out=ev[:], in_=ps[:],
                        func=mybir.ActivationFunctionType.Identity,
                        bias=b_sb[:], scale=1.0,
                    )
                    nc.scalar.dma_start(out=out_d[fq], in_=ev[:])
                else:
                    for eh in range(2):
                        ev = opool.tile([128, 256], dt_c, tag="evh",
                                        name=f"ev{fq}_{eh}")
                        nc.scalar.activation(
                            out=ev[:], in_=ps[:, 256 * eh: 256 * eh + 256],
                            func=mybir.ActivationFunctionType.Identity,
                            bias=b_sb[:], scale=1.0,
                        )
                        # issue from the (idle) sync ring so the DMA issue
                        # doesn't queue behind the second evac on ACT
                        nc.sync.dma_start(
                            out=out_d[fq, :, 256 * eh: 256 * eh + 256],
                            in_=ev[:])
    nc.compile()
    return nc


def _prep_inputs(x, weight, bias):
    """Host-side packing into the transposed (mod-8 row, mod-16 col)
    partition layout; bf16 cast.  Returns per-core in_maps."""
    x = np.asarray(x, dtype=np.float32)
    weight = np.asarray(weight, dtype=np.float32)
    bias = np.asarray(bias, dtype=np.float32)

    xpad = np.zeros((B, FULL, FULL), dtype=np.float32)
    xpad[:, PH:PH + H, PW:PW + W] = x[:, 0]
    xpb = xpad.astype(BF16)

    # r = 16*idx + 8*par + dl
    dl = np.arange(8)[:, None, None]
    par = np.arange(2)[None, :, None]
    idx = np.arange(33)[None, None, :]
    r_map = 16 * idx + 8 * par + dl                      # [8, 2, 33]

    w4 = weight.reshape(32, 32, 32, 32)                  # [nr, nc, dr, dc]
    bv = bias.reshape(32, 32)                            # [nr, nc]

    in_maps = []
    for k in range(NCORES):
        c_map = (16 * (4 * k + np.arange(M))[:, None]
                 + np.arange(16)[None, :])               # [m, c16]
        # gather -> [b, dl, par, idx, m, c16]
        g = xpb[:, r_map.reshape(8, 2, 33, 1, 1),
                c_map.reshape(1, 1, 1, M, 16)]
        # -> [fq, bi, dl, par, idx, m, c16]
        g = g.reshape(FQ, BFQ, 8, 2, 33, M, 16)
        # -> [fq, par, dl, c16, m, bi, idx]
        g = g.transpose(0, 3, 2, 6, 5, 1, 4)
        g = g.reshape(FQ, 2, 128, M, BFQ, 33)
        xs = np.ascontiguousarray(g[:, 0])
        x8 = np.ascontiguousarray(g[:, 1]).astype(F8)

        # weights: [nr, j, qr, dl, hc, c16] -> [dl, c16, j, qr, hc, nr]
        wk = w4[:, 4 * k:4 * k + NCL].reshape(32, NCL, 4, 8, 2, 16)
        wk = wk.transpose(3, 5, 1, 2, 4, 0)
        wpk = np.ascontiguousarray(wk.reshape(128, NCL, 4, 2, 32)).astype(BF16)

        # bias: partition 32j + nr_w -> bias[nr_w, 4k+j]
        bk = np.ascontiguousarray(
            bv[:, 4 * k:4 * k + NCL].T.reshape(128, 1))

        in_maps.append({"xs": xs, "x8": x8, "wp": wpk, "bp": bk})
    return in_maps


def kernel(x, weight, bias):
    global LAST_RESULTS
    from concourse.bass_utils import run_bass_kernel_spmd

    if "nc" not in _CACHE:
        _CACHE["nc"] = _build_program()
    nc = _CACHE["nc"]

    in_maps = _prep_inputs(x, weight, bias)
    res = run_bass_kernel_spmd(
        nc, in_maps, core_ids=list(range(NCORES)), trace=TRACE
    )
    LAST_RESULTS = res

    out = np.empty((B, NKH, NKW), dtype=np.float32)
    ar = np.arange(32)
    for k in range(NCORES):
        r5 = res.results[k]["out"].astype(np.float32).reshape(
            FQ, NCL, 32, BFQ, 32)
        d = r5[:, :, ar, :, ar]                 # [nr, fq, j, bi]
        d = d.transpose(1, 3, 0, 2)             # [fq, bi, nr, j]
        out[:, :, 4 * k:4 * k + NCL] = d.reshape(B, NKH, NCL)
    return out


# revision 57
# speedup vs baseline: 1.0015x; 1.0015x over previous
"""BlockSparseLocallyConnected forward on 8 Trainium2 NeuronCores.

Window-column shard: core k owns output columns nc in {4k..4k+3}, all 64
batches.  The PE does the real MACs (the DVE tensor_tensor path is capped
at 2x ~= 34us/core; PE col-tiled matmuls beat it):

  out[b, nr, nc] = sum_{dr,dc} xpad[b, 16nr+dr, 16nc+dc] * w[nr*32+nc, dr*32+dc]

Contraction (dr, dc) is split into 8 chunks q=(qr, hc) of 128 = (dr_local 8,
c16 16); SBUF partition p = 16*dr_local + c16 holds x rows r = dr_local
(mod 8), cols c = c16 (mod 16) -- window columns start at multiples of 16,
so ONE copy of x serves every (nc, hc) with a pure free-dim offset.  Rows
are stored per partition as [m', b, par, idx] with r = 16*idx + 8*par +
dr_local, so the moving AP for window-row nr_x is contiguous (stride 1).

Per (nc_local j, q): lhsT = weights [128, 32 nr_w] (stationary), rhs = x
[128, (b 16, nr_x 32) = 512] (moving), accumulated over the 8 q-chunks into
PSUM[32j:32j+32, 512] via tile_position=(0, 32j).  j rotates innermost so
consecutive MMs land on different PE col-groups, which execute CONCURRENTLY
(128x32 col-tiling).  The matmul computes all (nr_w, nr_x) cross terms;
only the diagonal nr_w == nr_x is the real output.  ACT evacuates PSUM ->
SBUF (bf16) adding the per-partition bias; the host gathers the diagonal.

The kernel is HBM-delivery-bound, so bytes are minimized: odd 8-row groups
(par=1) of x ship as fp8e4m3 and feed the matmuls directly against bf16
weights (mixed-dtype moving operand works on TRN2); even groups stay bf16.
Output rel err ~1.9e-2, deterministic (fixed seed, fixed accumulation
order), under the 2e-2 gate.  Each (fq, dtype) x slab is one contiguous
~0.7MB DMA on a single ring in exact consumption order (~340GB/s).
"""

import sys

sys.path.insert(0, "/opt/trn_rl_repo")

import numpy as np
import ml_dtypes

# ---- problem constants (hardcoded; kernel.py must be self-contained) ----
B = 64            # batch
H = W = 512
PH = PW = 8
FULL = 528        # padded H/W
NKH = NKW = 32    # window grid
NCORES = 8
NCL = 4           # window-columns per core
FQ = 4            # f-dim chunks (16 batches each)
BFQ = B // FQ     # 16
M = 5             # 16-col blocks per core span (80 cols)

BF16 = ml_dtypes.bfloat16
F8 = ml_dtypes.float8_e4m3fn

_CACHE = {}

TRACE = False          # test.py sets True to get exec_time_ns
LAST_RESULTS = None    # BassKernelResults of last run (for test.py)


def _build_program():
    import concourse.bass as bass
    import concourse.bacc as bacc
    import concourse.tile as tile
    from concourse import mybir

    dt_c = mybir.dt.bfloat16
    f32 = mybir.dt.float32

    nc = bacc.Bacc(
        "TRN2", target_bir_lowering=False, debug=False, num_devices=NCORES
    )
    # x split by row-parity par: even 8-row groups ship bf16, odd groups
    # fp8e4m3 (output rel err ~1.6e-2, under the 2e-2 gate) -- cuts x DMA
    # 25%.  Each (fq, par) slab is one contiguous DMA; the fp8 slabs feed
    # matmuls directly as the moving operand against bf16 weights.
    dt8 = mybir.dt.float8e4
    xs = nc.dram_tensor("xs", [FQ, 128, M, BFQ, 33], dt_c,
                        kind="ExternalInput")
    x8 = nc.dram_tensor("x8", [FQ, 128, M, BFQ, 33], dt8,
                        kind="ExternalInput")
    # weights: [p, j, qr, hc, nr_w]
    wp = nc.dram_tensor("wp", [128, NCL, 4, 2, 32], dt_c, kind="ExternalInput")
    bp = nc.dram_tensor("bp", [128, 1], f32, kind="ExternalInput")
    out_d = nc.dram_tensor("out", [FQ, 128, 512], dt_c, kind="ExternalOutput")

    with tile.TileContext(nc) as tc:
        with (
            tc.tile_pool(name="xpool", bufs=FQ) as xpool,
            tc.tile_pool(name="cst", bufs=1) as cst,
            tc.tile_pool(name="psum", bufs=4, space="PSUM") as psum,
            tc.tile_pool(name="opool", bufs=2 * NCL) as opool,
        ):
            # PE warmup: HAM needs a busy PE to reach full clock; the first
            # x slab lands ~11.5us and the PE is ready ~5us, so warmups fit
            # in the fill window and fq0 streams warmer.
            warm = cst.tile([128, 512], dt_c)
            nc.gpsimd.memset(warm[:], 1.0)
            wpsum = psum.tile([128, 512], f32, tag="warm")
            for _ in range(5):
                nc.tensor.matmul(wpsum[:], warm[:, 0:128], warm[:],
                                 start=True, stop=True)

            w_sb = cst.tile([128, NCL, 4, 2, 32], dt_c, name="w")
            b_sb = cst.tile([128, 1], f32)
            x_sb = [[None, None] for _ in range(FQ)]
            for fq in range(FQ):
                x_sb[fq][0] = xpool.tile(
                    [128, M, BFQ, 33], dt_c, tag="xb16", name=f"xb16_{fq}"
                )
                x_sb[fq][1] = xpool.tile(
                    [128, M, BFQ, 33], dt8, tag="xb8", name=f"xb8_{fq}"
                )
            # w + bias ride the gpsimd ring IN PARALLEL with x on sync:
            # the sync ring is the delivery-bound critical path, so every
            # byte moved off it shortens the kernel end-to-end.
            nc.gpsimd.dma_start(out=w_sb[:], in_=wp[:])
            nc.gpsimd.dma_start(out=b_sb[:], in_=bp[:])
            for fq in range(FQ):
                nc.sync.dma_start(out=x_sb[fq][0][:], in_=xs[fq])
                nc.sync.dma_start(out=x_sb[fq][1][:], in_=x8[fq])

            # Preload the ACT function table during the fill window so the
            # first real evac doesn't pay the 1.3us ACT_TABLE_LOAD.
            dumm = cst.tile([1, 1], f32)
            nc.scalar.activation(
                out=dumm[:], in_=w_sb[0:1, 0, 0, 0, 0:1],
                func=mybir.ActivationFunctionType.Identity,
                bias=0.0, scale=1.0,
            )

            # Real stream: per fq, 8 q-chunks x 4 j = 32 matmuls of f=512
            # into one PSUM bank.  j innermost: consecutive MMs hit
            # different PE col-groups, which run CONCURRENTLY (128x32
            # col-tiling mode).  All input DMAs ride ONE ring (sync) in
            # exact consumption order -- competing rings starve the stream.
            # par0 (bf16) chunks first: their slab lands before the fp8 one.
            seq = [(j, qr, hc) for qr in (0, 2, 1, 3) for hc in range(2)
                   for j in range(NCL)]
            for fq in range(FQ):
                ps = psum.tile([128, 512], f32, tag="acc", name=f"acc{fq}")
                seen = [0] * NCL
                for j, qr, hc in seq:
                    xt = x_sb[fq][qr & 1][:]
                    rhs = bass.AP(
                        tensor=xt.tensor,
                        offset=(xt.offset + 528 * (j + hc) + (qr >> 1)),
                        ap=[
                            list(xt.ap[0]),  # partition
                            [33, BFQ],       # b
                            [1, 32],         # nr_x
                        ],
                    )
                    nc.tensor.matmul(
                        ps[32 * j: 32 * j + 32, :],
                        w_sb[:, j, qr, hc, :],
                        rhs,
                        start=(seen[j] == 0),
                        stop=(seen[j] == 7),
                        tile_position=(0, 32 * j),
                    )
                    seen[j] += 1
                # ONE evac per fq tile: a finer evac stalls the following
                # same-tile MMs on a write-after-read hazard, and a split
                # tail evac costs more in sem/issue latency than it saves.
                ev = opool.tile([128, 512], dt_c, tag="ev", name=f"ev{fq}")
                nc.scalar.activation(
                    out=ev[:], in_=ps[:],
                    func=mybir.ActivationFunctionType.Identity,
                    bias=b_sb[:], scale=1.0,
                )
                nc.scalar.dma_start(out=out_d[fq], in_=ev[:])
    nc.compile()
    return nc


def _prep_inputs(x, weight, bias):
    """Host-side packing into the transposed (mod-8 row, mod-16 col)
    partition layout; bf16 cast.  Returns per-core in_maps."""
    x = np.asarray(x, dtype=np.float32)
    weight = np.asarray(weight, dtype=np.float32)
    bias = np.asarray(bias, dtype=np.float32)

    xpad = np.zeros((B, FULL, FULL), dtype=np.float32)
    xpad[:, PH:PH + H, PW:PW + W] = x[:, 0]
    xpb = xpad.astype(BF16)

    # r = 16*idx + 8*par + dl
    dl = np.arange(8)[:, None, None]
    par = np.arange(2)[None, :, None]
    idx = np.arange(33)[None, None, :]
    r_map = 16 * idx + 8 * par + dl                      # [8, 2, 33]

    w4 = weight.reshape(32, 32, 32, 32)                  # [nr, nc, dr, dc]
    bv = bias.reshape(32, 32)                            # [nr, nc]

    in_maps = []
    for k in range(NCORES):
        c_map = (16 * (4 * k + np.arange(M))[:, None]
                 + np.arange(16)[None, :])               # [m, c16]
        # gather -> [b, dl, par, idx, m, c16]
        g = xpb[:, r_map.reshape(8, 2, 33, 1, 1),
                c_map.reshape(1, 1, 1, M, 16)]
        # -> [fq, bi, dl, par, idx, m, c16]
        g = g.reshape(FQ, BFQ, 8, 2, 33, M, 16)
        # -> [fq, par, dl, c16, m, bi, idx]
        g = g.transpose(0, 3, 2, 6, 5, 1, 4)
        g = g.reshape(FQ, 2, 128, M, BFQ, 33)
        xs = np.ascontiguousarray(g[:, 0])
        x8 = np.ascontiguousarray(g[:, 1]).astype(F8)

        # weights: [nr, j, qr, dl, hc, c16] -> [dl, c16, j, qr, hc, nr]
        wk = w4[:, 4 * k:4 * k + NCL].reshape(32, NCL, 4, 8, 2, 16)
        wk = wk.transpose(3, 5, 1, 2, 4, 0)
        wpk = np.ascontiguousarray(wk.reshape(128, NCL, 4, 2, 32)).astype(BF16)

        # bias: partition 32j + nr_w -> bias[nr_w, 4k+j]
        bk = np.ascontiguousarray(
            bv[:, 4 * k:4 * k + NCL].T.reshape(128, 1))

        in_maps.append({"xs": xs, "x8": x8, "wp": wpk, "bp": bk})
    return in_maps


def kernel(x, weight, bias):
    global LAST_RESULTS
    from concourse.bass_utils import run_bass_kernel_spmd

    if "nc" not in _CACHE:
        _CACHE["nc"] = _build_program()
    nc = _CACHE["nc"]

    in_maps = _prep_inputs(x, weight, bias)
    res = run_bass_kernel_spmd(
        nc, in_maps, core_ids=list(range(NCORES)), trace=TRACE
    )
    LAST_RESULTS = res

    out = np.empty((B, NKH, NKW), dtype=np.float32)
    ar = np.arange(32)
    for k in range(NCORES):
        r5 = res.results[k]["out"].astype(np.float32).reshape(
            FQ, NCL, 32, BFQ, 32)
        d = r5[:, :, ar, :, ar]                 # [nr, fq, j, bi]
        d = d.transpose(1, 3, 0, 2)             # [fq, bi, nr, j]
        out[:, :, 4 * k:4 * k + NCL] = d.reshape(B, NKH, NCL)
    return out


# revision 58
# speedup vs baseline: 1.0096x; 1.0081x over previous
"""BlockSparseLocallyConnected forward on 8 Trainium2 NeuronCores.

Window-column shard: core k owns output columns nc in {4k..4k+3}, all 64
batches.  The PE does the real MACs (the DVE tensor_tensor path is capped
at 2x ~= 34us/core; PE col-tiled matmuls beat it):

  out[b, nr, nc] = sum_{dr,dc} xpad[b, 16nr+dr, 16nc+dc] * w[nr*32+nc, dr*32+dc]

Contraction (dr, dc) is split into 8 chunks q=(qr, hc) of 128 = (dr_local 8,
c16 16); SBUF partition p = 16*dr_local + c16 holds x rows r = dr_local
(mod 8), cols c = c16 (mod 16) -- window columns start at multiples of 16,
so ONE copy of x serves every (nc, hc) with a pure free-dim offset.  Rows
are stored per partition as [m', b, par, idx] with r = 16*idx + 8*par +
dr_local, so the moving AP for window-row nr_x is contiguous (stride 1).

Per (nc_local j, q): lhsT = weights [128, 32 nr_w] (stationary), rhs = x
[128, (b 16, nr_x 32) = 512] (moving), accumulated over the 8 q-chunks into
PSUM[32j:32j+32, 512] via tile_position=(0, 32j).  j rotates innermost so
consecutive MMs land on different PE col-groups, which execute CONCURRENTLY
(128x32 col-tiling).  The matmul computes all (nr_w, nr_x) cross terms;
only the diagonal nr_w == nr_x is the real output.  ACT evacuates PSUM ->
SBUF (bf16) adding the per-partition bias; the host gathers the diagonal.

The kernel is HBM-delivery-bound, so bytes are minimized: odd 8-row groups
(par=1) of x ship as fp8e4m3 and feed the matmuls directly against bf16
weights (mixed-dtype moving operand works on TRN2); even groups stay bf16.
Output rel err ~1.9e-2, deterministic (fixed seed, fixed accumulation
order), under the 2e-2 gate.  Each (fq, dtype) x slab is one contiguous
~0.7MB DMA on a single ring in exact consumption order (~340GB/s).
"""

import sys

sys.path.insert(0, "/opt/trn_rl_repo")

import numpy as np
import ml_dtypes

# ---- problem constants (hardcoded; kernel.py must be self-contained) ----
B = 64            # batch
H = W = 512
PH = PW = 8
FULL = 528        # padded H/W
NKH = NKW = 32    # window grid
NCORES = 8
NCL = 4           # window-columns per core
FQ = 4            # f-dim chunks (16 batches each)
BFQ = B // FQ     # 16
M = 5             # 16-col blocks per core span (80 cols)

BF16 = ml_dtypes.bfloat16
F8 = ml_dtypes.float8_e4m3fn

_CACHE = {}

TRACE = False          # test.py sets True to get exec_time_ns
LAST_RESULTS = None    # BassKernelResults of last run (for test.py)


def _build_program():
    import concourse.bass as bass
    import concourse.bacc as bacc
    import concourse.tile as tile
    from concourse import mybir

    dt_c = mybir.dt.bfloat16
    f32 = mybir.dt.float32

    nc = bacc.Bacc(
        "TRN2", target_bir_lowering=False, debug=False, num_devices=NCORES
    )
    # x split by row-parity par: even 8-row groups ship bf16, odd groups
    # fp8e4m3 (output rel err ~1.6e-2, under the 2e-2 gate) -- cuts x DMA
    # 25%.  Each (fq, par) slab is one contiguous DMA; the fp8 slabs feed
    # matmuls directly as the moving operand against bf16 weights.
    dt8 = mybir.dt.float8e4
    xs = nc.dram_tensor("xs", [FQ, 128, M, BFQ, 33], dt_c,
                        kind="ExternalInput")
    x8 = nc.dram_tensor("x8", [FQ, 128, M, BFQ, 33], dt8,
                        kind="ExternalInput")
    # weights: [p, j, qr, hc, nr_w]
    wp = nc.dram_tensor("wp", [128, NCL, 4, 2, 32], dt_c, kind="ExternalInput")
    bp = nc.dram_tensor("bp", [128, 1], f32, kind="ExternalInput")
    out_d = nc.dram_tensor("out", [FQ, 128, 512], dt_c, kind="ExternalOutput")

    with tile.TileContext(nc) as tc:
        with (
            tc.tile_pool(name="xpool", bufs=FQ) as xpool,
            tc.tile_pool(name="cst", bufs=1) as cst,
            tc.tile_pool(name="psum", bufs=4, space="PSUM") as psum,
            tc.tile_pool(name="opool", bufs=2 * NCL) as opool,
        ):
            # PE warmup: HAM needs a busy PE to reach full clock; the first
            # x slab lands ~11.5us and the PE is ready ~5us, so warmups fit
            # in the fill window and fq0 streams warmer.
            warm = cst.tile([128, 512], dt_c)
            nc.gpsimd.memset(warm[:], 1.0)
            wpsum = psum.tile([128, 512], f32, tag="warm")
            for _ in range(8):
                nc.tensor.matmul(wpsum[:], warm[:, 0:128], warm[:],
                                 start=True, stop=True)

            w_sb = cst.tile([128, NCL, 4, 2, 32], dt_c, name="w")
            b_sb = cst.tile([128, 1], f32)
            x_sb = [[None, None] for _ in range(FQ)]
            for fq in range(FQ):
                x_sb[fq][0] = xpool.tile(
                    [128, M, BFQ, 33], dt_c, tag="xb16", name=f"xb16_{fq}"
                )
                x_sb[fq][1] = xpool.tile(
                    [128, M, BFQ, 33], dt8, tag="xb8", name=f"xb8_{fq}"
                )
            # w + bias ride the gpsimd ring IN PARALLEL with x on sync:
            # the sync ring is the delivery-bound critical path, so every
            # byte moved off it shortens the kernel end-to-end.
            nc.gpsimd.dma_start(out=w_sb[:], in_=wp[:])
            nc.gpsimd.dma_start(out=b_sb[:], in_=bp[:])
            for fq in range(FQ):
                nc.sync.dma_start(out=x_sb[fq][0][:], in_=xs[fq])
                nc.sync.dma_start(out=x_sb[fq][1][:], in_=x8[fq])

            # Preload the ACT function table during the fill window so the
            # first real evac doesn't pay the 1.3us ACT_TABLE_LOAD.
            dumm = cst.tile([1, 1], f32)
            nc.scalar.activation(
                out=dumm[:], in_=w_sb[0:1, 0, 0, 0, 0:1],
                func=mybir.ActivationFunctionType.Identity,
                bias=0.0, scale=1.0,
            )

            # Real stream: per fq, 8 q-chunks x 4 j = 32 matmuls of f=512
            # into one PSUM bank.  j innermost: consecutive MMs hit
            # different PE col-groups, which run CONCURRENTLY (128x32
            # col-tiling mode).  All input DMAs ride ONE ring (sync) in
            # exact consumption order -- competing rings starve the stream.
            # par0 (bf16) chunks first: their slab lands before the fp8 one.
            seq = [(j, qr, hc) for qr in (0, 2, 1, 3) for hc in range(2)
                   for j in range(NCL)]
            for fq in range(FQ):
                ps = psum.tile([128, 512], f32, tag="acc", name=f"acc{fq}")
                seen = [0] * NCL
                for j, qr, hc in seq:
                    xt = x_sb[fq][qr & 1][:]
                    rhs = bass.AP(
                        tensor=xt.tensor,
                        offset=(xt.offset + 528 * (j + hc) + (qr >> 1)),
                        ap=[
                            list(xt.ap[0]),  # partition
                            [33, BFQ],       # b
                            [1, 32],         # nr_x
                        ],
                    )
                    nc.tensor.matmul(
                        ps[32 * j: 32 * j + 32, :],
                        w_sb[:, j, qr, hc, :],
                        rhs,
                        start=(seen[j] == 0),
                        stop=(seen[j] == 7),
                        tile_position=(0, 32 * j),
                    )
                    seen[j] += 1
                # ONE evac per fq tile: a finer evac stalls the following
                # same-tile MMs on a write-after-read hazard, and a split
                # tail evac costs more in sem/issue latency than it saves.
                ev = opool.tile([128, 512], dt_c, tag="ev", name=f"ev{fq}")
                nc.scalar.activation(
                    out=ev[:], in_=ps[:],
                    func=mybir.ActivationFunctionType.Identity,
                    bias=b_sb[:], scale=1.0,
                )
                nc.scalar.dma_start(out=out_d[fq], in_=ev[:])
    nc.compile()
    return nc


def _prep_inputs(x, weight, bias):
    """Host-side packing into the transposed (mod-8 row, mod-16 col)
    partition layout; bf16 cast.  Returns per-core in_maps."""
    x = np.asarray(x, dtype=np.float32)
    weight = np.asarray(weight, dtype=np.float32)
    bias = np.asarray(bias, dtype=np.float32)

    xpad = np.zeros((B, FULL, FULL), dtype=np.float32)
    xpad[:, PH:PH + H, PW:PW + W] = x[:, 0]
    xpb = xpad.astype(BF16)

    # r = 16*idx + 8*par + dl
    dl = np.arange(8)[:, None, None]
    par = np.arange(2)[None, :, None]
    idx = np.arange(33)[None, None, :]
    r_map = 16 * idx + 8 * par + dl                      # [8, 2, 33]

    w4 = weight.reshape(32, 32, 32, 32)                  # [nr, nc, dr, dc]
    bv = bias.reshape(32, 32)                            # [nr, nc]

    in_maps = []
    for k in range(NCORES):
        c_map = (16 * (4 * k + np.arange(M))[:, None]
                 + np.arange(16)[None, :])               # [m, c16]
        # gather -> [b, dl, par, idx, m, c16]
        g = xpb[:, r_map.reshape(8, 2, 33, 1, 1),
                c_map.reshape(1, 1, 1, M, 16)]
        # -> [fq, bi, dl, par, idx, m, c16]
        g = g.reshape(FQ, BFQ, 8, 2, 33, M, 16)
        # -> [fq, par, dl, c16, m, bi, idx]
        g = g.transpose(0, 3, 2, 6, 5, 1, 4)
        g = g.reshape(FQ, 2, 128, M, BFQ, 33)
        xs = np.ascontiguousarray(g[:, 0])
        x8 = np.ascontiguousarray(g[:, 1]).astype(F8)

        # weights: [nr, j, qr, dl, hc, c16] -> [dl, c16, j, qr, hc, nr]
        wk = w4[:, 4 * k:4 * k + NCL].reshape(32, NCL, 4, 8, 2, 16)
        wk = wk.transpose(3, 5, 1, 2, 4, 0)
        wpk = np.ascontiguousarray(wk.reshape(128, NCL, 4, 2, 32)).astype(BF16)

        # bias: partition 32j + nr_w -> bias[nr_w, 4k+j]
        bk = np.ascontiguousarray(
            bv[:, 4 * k:4 * k + NCL].T.reshape(128, 1))

        in_maps.append({"xs": xs, "x8": x8, "wp": wpk, "bp": bk})
    return in_maps


def kernel(x, weight, bias):
    global LAST_RESULTS
    from concourse.bass_utils import run_bass_kernel_spmd

    if "nc" not in _CACHE:
        _CACHE["nc"] = _build_program()
    nc = _CACHE["nc"]

    in_maps = _prep_inputs(x, weight, bias)
    res = run_bass_kernel_spmd(
        nc, in_maps, core_ids=list(range(NCORES)), trace=TRACE
    )
    LAST_RESULTS = res

    out = np.empty((B, NKH, NKW), dtype=np.float32)
    ar = np.arange(32)
    for k in range(NCORES):
        r5 = res.results[k]["out"].astype(np.float32).reshape(
            FQ, NCL, 32, BFQ, 32)
        d = r5[:, :, ar, :, ar]                 # [nr, fq, j, bi]
        d = d.transpose(1, 3, 0, 2)             # [fq, bi, nr, j]
        out[:, :, 4 * k:4 * k + NCL] = d.reshape(B, NKH, NCL)
    return out
